# revision 42
# baseline (speedup 1.0000x reference)
"""FFM layer kernel for 8 Trainium2 NeuronCores — fp8 DoubleRow version.

Math (reference): x[B,39] = 13 dense cols + 26 sparse index cols (ints 0..99
stored as f32).  inputs[B,2613] = [dense | one_hot(sparse)], then
  linear = inputs @ w.T + b
  field  = einsum('bn,nfk->bfk', inputs, v)        # [B,39,16]
  cross  = 0.5*sum_k((sum_f field)^2 - sum_f field^2)
  out    = sigmoid(linear + cross)

Strategy: data-parallel over batch, 2048 rows/core.  On each core the one-hot
matrix is built on-device and used as the stationary operand of fp8 DoubleRow
matmuls (256-deep contraction per pass, 2x PE throughput):
  psum[128b, 658] = sum_pairs oh[128f,2,128b].T @ vp_pair[128f,2,658]
Columns: 0..623 = field (f*16+k) scaled 16x, 624..639 = V1hi ~ sum_f v,
640..655 = V1res (16x residual), 656..657 = w_hi/w_res.  The dedicated V1/w
columns give a near-exact s[b,k] and linear part so fp8 quantization of v
only enters through sum_f field^2 (rel err ~3e-3, budget 2e-2).

Device tricks (all found the hard way):
 - DVE/GpSimd have no fast store path for 1-byte dtypes (~5 cyc/elem
   microcode), so the one-hot is built in fp16: is_equal writes 1.0 =
   0x3C00, whose HIGH byte 0x3C == e4m3(1.5).  The matmul reads a stride-2
   fp8 view of the high bytes; the 1.5x is divided out of the stationary.
 - The host ships idx PRE-SUBTRACTED by the vocab ramp, so one is_equal
   against the constant 0 covers many chunks per instruction (amortizing
   the ~140-cycle DVE instruction overhead), instead of one instruction per
   chunk with a per-partition ramp scalar.
 - idx is laid out in column blocks matching the build-pass schedule and
   all idx DMAs are sequenced on the sync queue, so block 0 (360KB) lands
   ~2us in and each later block arrives before its build pass needs it.
 - A 22nd all-zero chunk pads the 21 feature chunks to 11 full DoubleRow
   pairs (a lone non-DoubleRow fp8 matmul measured ~1.8 cyc/col).
 - GpSimd tensor ops are microcode-slow; the whole build runs on the DVE as
   wide column passes ([1,2,2,4,4,3] batch tiles) that stay ahead of PE
   consumption, with epilogue vector ops interleaved between passes at
   points chosen so PSUM tiles recycle before the PE needs their slot.

Feature rows: [1s row (bias via w cols) | 13 x_hi | 13 x_res | pad to 32 |
26*100 one-hot | zero tail].  fp32 PSUM.  Epilogue: s = V1hi_col +
V1res_col/16 (DVE), sq via scalar Square+accumulate, sigmoid on scalar.
Throwaway fp8 warmup matmuls release the HAM clock throttle during the DMA
head.
"""

import sys

sys.path.insert(0, "/opt/trn_rl_repo")

import numpy as np
import ml_dtypes

import concourse.tile as tile
from concourse import bacc, mybir
from concourse.bass_utils import run_bass_kernel_spmd

N_CORES = 8
B_FULL = 16384
BC = B_FULL // N_CORES  # 2048 rows per core
P = 128
N_DENSE = 13
N_SPARSE = 26
SPARSE_DIM = 100
N_FIELD = 39
K_DIM = 16
NCHUNK = 22             # 21 feature chunks + 1 zero pad chunk
NPAIR = 11              # all contraction passes are DoubleRow pairs
RTOT = NCHUNK * P       # 2816 padded feature rows
SP0 = 32                # first one-hot row
NFEAT_END = SP0 + N_SPARSE * SPARSE_DIM  # 2632
NFK = N_FIELD * K_DIM   # 624
COLS = NFK + 2 * K_DIM + 2  # 658
CSPLIT = 512
VSCALE = 16.0           # field cols store 16*v; sq_raw = 256*sq
LHS = 1.5               # one-hot lhs cells read e4m3(1.5); vp pre-divided

NCH_D = 21              # chunks actually shipped (pad chunk 21 is memset)
# build passes: batch-tile ranges; idx ships in matching column blocks.
# Small early blocks so the DMA stream stays ahead of the PE's 3us/bt pace.
PASS_BTS = [(0, 1), (1, 2), (2, 3), (3, 5), (5, 9), (9, 13), (13, 16)]
# chunk sub-ranges per pass (delivery granularity for the PE).  NOTE: the
# chunk count of each sub must be EVEN (or 1) or the DVE drops from 2
# elem/cycle to 1 (2x_2P needs the major non-unit dim even).
CSUBS = [(0, 2), (2, 12), (12, 20), (20, 21)]

F8 = mybir.dt.float8e4
F16 = mybir.dt.float16
F32 = mybir.dt.float32
I8 = mybir.dt.int8
E4NP = ml_dtypes.float8_e4m3

_prog_cache = {}


def _q8(a):
    """Round-trip through TRN e4m3 (numpy f32 in/out)."""
    return np.clip(a, -240, 240).astype(E4NP).astype(np.float32)


def _build_program(bc):
    """One SPMD program for a batch slice of `bc` rows (all cores identical)."""
    nbt = bc // P
    idxtot = NCH_D * bc

    nc = bacc.Bacc("TRN2", target_bir_lowering=False, debug=False)
    idx_d = nc.declare_dram_parameter("idxs", [P, idxtot], I8, isOutput=False)
    xdn_d = nc.declare_dram_parameter("xdn", [SP0, bc], F16, isOutput=False)
    vp_d = nc.declare_dram_parameter(
        "vperm", [P, NCH_D, COLS], F8, isOutput=False)
    y_d = nc.declare_dram_parameter("y", [P, nbt], F32, isOutput=True)

    DR = mybir.MatmulPerfMode.DoubleRow

    with tile.TileContext(nc) as tc:
        with (
            tc.tile_pool(name="pers", bufs=1) as pers,
            tc.tile_pool(name="psum", bufs=4, space="PSUM") as psum,
            tc.tile_pool(name="epi", bufs=3) as epi,
        ):
            # one-hot stationary, fp16; matmuls read stride-2 fp8 view
            oh_all = pers.tile([P, NCHUNK, bc], F16, tag="oh", name="oh")
            oh8 = oh_all[:].bitcast(F8).rearrange("p c (b t) -> p c b t", t=2)

            y_all = pers.tile([P, nbt], F32, tag="yall")
            vp_all = pers.tile([P, NCHUNK, COLS], F8, tag="vp")
            # per-pass idx blocks, flat [P, NCH_D*w] so each DMA moves one
            # long contiguous run per partition; chunk-major inside
            idxb = []
            idxv = []
            for pi, (bt0, bt1) in enumerate(PASS_BTS):
                w = (bt1 - bt0) * P
                t = pers.tile([P, NCH_D * w], I8, tag=f"idx{pi}")
                idxb.append(t)
                idxv.append(t[:].rearrange("p (c w) -> p c w", w=w))

            xdn_t = pers.tile([SP0, bc], F16, tag="xdn")
            # deadline-ordered input DMAs: sync carries idx block 0, the
            # bulk of vperm, then the later idx blocks; scalar carries the
            # small early tensors (xdn + first vperm pair)
            nc.scalar.dma_start(xdn_t[:], xdn_d[:])
            nc.scalar.dma_start(vp_all[:, 0:2, :], vp_d[:, 0:2, :])
            offs = []
            off = 0
            for bt0, bt1 in PASS_BTS:
                offs.append(off)
                off += NCH_D * (bt1 - bt0) * P

            def load_idx(pi):
                w = NCH_D * (PASS_BTS[pi][1] - PASS_BTS[pi][0]) * P
                nc.sync.dma_start(idxb[pi][:], idx_d[:, offs[pi]:offs[pi] + w])

            load_idx(0)
            nc.sync.dma_start(vp_all[:, 2:8, :], vp_d[:, 2:8, :])
            nc.sync.dma_start(vp_all[:, 8:15, :], vp_d[:, 8:15, :])
            nc.sync.dma_start(vp_all[:, 15:NCH_D, :], vp_d[:, 15:NCH_D, :])
            for pi in range(1, len(PASS_BTS)):
                load_idx(pi)

            # PE warmup: throwaway fp8 DoubleRow matmuls release the HAM
            # clock throttle and cover the DMA head + first build pass.
            # The warm tiles are fp16-declared (DVE memsets 2-byte dtypes
            # fast, and the DVE queue starts ~1.5us before gpsimd's) and
            # bitcast to fp8 for the matmuls.
            wz16 = pers.tile([P, 2, 8], F16, tag="wz16")
            wz512 = pers.tile([P, 2, 256], F16, tag="wz512")
            nc.gpsimd.memset(wz16[:], 0.0)
            nc.gpsimd.memset(wz512[:], 0.0)
            wz16v = wz16[:].bitcast(F8)
            wz512v = wz512[:].bitcast(F8)
            # pad chunk 21 (one-hot + stationary) is all zeros: memset once
            # on the idle gpsimd engine instead of shipping it
            nc.gpsimd.memset(oh_all[:, NCH_D, :], 0.0)
            nc.gpsimd.memset(vp_all[:, NCH_D, :], 0.0)
            wps = psum.tile([P, COLS], F32, tag="ps", name="warmps")
            for _ in range(10):
                nc.tensor.matmul(wps[0:16, 0:512], wz16v, wz512v,
                                 start=True, stop=True, perf_mode=DR)
            for _ in range(50):
                nc.tensor.matmul(wps[0:16, 0:64], wz16v, wz512v[:, :, 0:64],
                                 start=True, stop=True, perf_mode=DR)

            def build_sub(pi, c0, c1):
                """is_equal(idx_block - 0) over chunks [c0,c1) of pass pi."""
                bt0, bt1 = PASS_BTS[pi]
                j0 = bt0 * P
                w = (bt1 - bt0) * P
                nc.vector.tensor_scalar(
                    out=oh_all[:, c0:c1, j0:j0 + w],
                    in0=idxv[pi][:, c0:c1, :],
                    scalar1=0.0, scalar2=None,
                    op0=mybir.AluOpType.is_equal,
                )
                if c0 == 0:
                    # head rows of chunk 0: bias/x_hi/x_res, prebaked fp16
                    # bit patterns; overwrites the junk the compare wrote
                    nc.vector.tensor_copy(
                        oh_all[0:SP0, 0, j0:j0 + w], xdn_t[:, j0:j0 + w])

            def issue_matmuls(bt):
                ps = psum.tile([P, COLS], F32, tag="ps")
                b0, b1 = bt * P, (bt + 1) * P
                for j in range(NPAIR):
                    lhs = oh8[:, 2 * j:2 * j + 2, b0:b1, 1]
                    st = (j == 0)
                    sp = (j == NPAIR - 1)
                    nc.tensor.matmul(
                        ps[:, 0:CSPLIT], lhs,
                        vp_all[:, 2 * j:2 * j + 2, 0:CSPLIT],
                        start=st, stop=sp, perf_mode=DR)
                    nc.tensor.matmul(
                        ps[:, CSPLIT:COLS], lhs,
                        vp_all[:, 2 * j:2 * j + 2, CSPLIT:COLS],
                        start=st, stop=sp, perf_mode=DR)
                # scalar-engine Square+accumulate over the field columns
                sq_scr = epi.tile([P, NFK], F32, tag="sqscr")
                sqsum = epi.tile([P, 1], F32, tag="sqsum")
                nc.scalar.activation(
                    out=sq_scr[:], in_=ps[:, 0:NFK],
                    func=mybir.ActivationFunctionType.Square,
                    accum_out=sqsum[:],
                )
                return ps, sqsum

            def issue_epi(bt, ps, sqsum):
                """DVE combine ops + scalar s2/sigmoid for one batch tile."""
                sres = epi.tile([P, K_DIM], F32, tag="sres")
                nc.vector.tensor_scalar(
                    out=sres[:], in0=ps[:, NFK + K_DIM:NFK + 2 * K_DIM],
                    scalar1=1.0 / VSCALE, scalar2=None,
                    op0=mybir.AluOpType.mult,
                )
                s_t = epi.tile([P, K_DIM], F32, tag="s")
                nc.vector.tensor_tensor(
                    out=s_t[:], in0=sres[:], in1=ps[:, NFK:NFK + K_DIM],
                    op=mybir.AluOpType.add,
                )
                lin = epi.tile([P, 1], F32, tag="lin")
                nc.vector.tensor_scalar(
                    out=lin[:], in0=ps[:, COLS - 1:COLS],
                    scalar1=1.0 / VSCALE, scalar2=ps[:, COLS - 2:COLS - 1],
                    op0=mybir.AluOpType.mult,
                    op1=mybir.AluOpType.add,
                )
                b2 = epi.tile([P, 1], F32, tag="b2")
                nc.vector.tensor_scalar(
                    out=b2[:], in0=sqsum[:],
                    scalar1=-0.5 / (VSCALE * VSCALE), scalar2=lin[:],
                    op0=mybir.AluOpType.mult,
                    op1=mybir.AluOpType.add,
                )
                s2_scr = epi.tile([P, K_DIM], F32, tag="s2scr")
                s2sum = epi.tile([P, 1], F32, tag="s2sum")
                nc.scalar.activation(
                    out=s2_scr[:], in_=s_t[:],
                    func=mybir.ActivationFunctionType.Square,
                    accum_out=s2sum[:],
                )
                nc.scalar.activation(
                    out=y_all[:, bt:bt + 1], in_=s2sum[:],
                    func=mybir.ActivationFunctionType.Sigmoid,
                    scale=0.5, bias=b2[:],
                )

            # DVE schedule: build sub-passes with epilogue batches woven in
            # so each psum tile is released before the PE reuses its slot
            bt_state = {}

            def deliver(pi):
                bt0, bt1 = PASS_BTS[pi]
                for bt in range(bt0, bt1):
                    bt_state[bt] = issue_matmuls(bt)

            def ep(bt):
                issue_epi(bt, *bt_state.pop(bt))

            for pi in (0, 1, 2, 3):
                for c0, c1 in CSUBS:
                    build_sub(pi, c0, c1)
                deliver(pi)
            ep(0)
            ep(1)
            build_sub(4, *CSUBS[0])
            ep(2)
            build_sub(4, *CSUBS[1])
            ep(3)
            build_sub(4, *CSUBS[2])
            build_sub(4, *CSUBS[3])
            deliver(4)
            ep(4)
            build_sub(5, *CSUBS[0])
            ep(5)
            build_sub(5, *CSUBS[1])
            ep(6)
            build_sub(5, *CSUBS[2])
            build_sub(5, *CSUBS[3])
            deliver(5)
            ep(7)
            nc.sync.dma_start(y_d[:, 0:8], y_all[:, 0:8])
            for c0, c1 in CSUBS:
                build_sub(6, c0, c1)
            deliver(6)
            for bt in range(8, nbt - 1):
                ep(bt)
            nc.sync.dma_start(y_d[:, 8:nbt - 1], y_all[:, 8:nbt - 1])
            ep(nbt - 1)
            nc.sync.dma_start(y_d[:, nbt - 1:nbt], y_all[:, nbt - 1:nbt])

    nc.compile()
    return nc


def _get_program(bc):
    if bc not in _prog_cache:
        _prog_cache[bc] = _build_program(bc)
    return _prog_cache[bc]


def _prep_shared(w_weight, w_bias, v):
    """vperm[128, 22, 658] e4m3 plus row->field maps (same on every core).

    The one-hot lhs cells read 1.5 (fp16 1.0's high byte as e4m3), so every
    stationary value is divided by LHS (=1.5) before quantization; the psum
    then carries the intended products.
    """
    v2 = v.reshape(2613, NFK)            # col = f*16 + k
    V1 = v.sum(axis=1) / LHS             # [2613, 16]
    V1hi = _q8(V1)
    V1res = _q8(VSCALE * (V1 - V1hi))
    w = w_weight[0] / LHS
    whi = _q8(w)
    wres = _q8(VSCALE * (w - whi))

    vp = np.zeros((RTOT, COLS), np.float32)

    def fill(rows, n0, n1):
        vp[rows, 0:NFK] = _q8(VSCALE / LHS * v2[n0:n1])
        vp[rows, NFK:NFK + K_DIM] = V1hi[n0:n1]
        vp[rows, NFK + K_DIM:NFK + 2 * K_DIM] = V1res[n0:n1]
        vp[rows, COLS - 2] = whi[n0:n1]
        vp[rows, COLS - 1] = wres[n0:n1]

    fill(slice(1, 1 + N_DENSE), 0, N_DENSE)          # x_hi rows
    fill(slice(14, 14 + N_DENSE), 0, N_DENSE)        # x_res rows
    fill(slice(SP0, NFEAT_END), N_DENSE, 2613)       # one-hot rows
    b = float(w_bias[0]) / LHS
    bhi = _q8(np.float32(b))
    vp[0, COLS - 2] = bhi
    vp[0, COLS - 1] = _q8(np.float32(VSCALE * (b - bhi)))
    vp8 = np.ascontiguousarray(
        vp.astype(E4NP).reshape(NCHUNK, P, COLS)[:NCH_D].transpose(1, 0, 2))

    r = np.arange(RTOT)
    in_sparse = (r >= SP0) & (r < NFEAT_END)
    off = np.where(in_sparse, (r - SP0) % SPARSE_DIM, 0)
    s_of_r = np.where(in_sparse, (r - SP0) // SPARSE_DIM, -1)
    return vp8, off, s_of_r, in_sparse


def _prep_core(x_core, off, s_of_r, in_sparse):
    """Per-core idxs[128, 22*bc] int8 (pre-subtracted, block-major) and
    dense xdn[32, bc] fp16-bit-packed."""
    bc = x_core.shape[0]
    # idxsub[r, b] = idx - vocab_offset for one-hot rows (0 iff hot), else 1
    idxsub = np.ones((RTOT, bc), np.int8)
    cols = (N_DENSE + s_of_r[in_sparse]).astype(np.int64)
    idxsub[in_sparse] = (
        x_core[:, cols].T.astype(np.int16)
        - off[in_sparse][:, None]).astype(np.int8)
    cm = idxsub.reshape(NCHUNK, P, bc)[:NCH_D]   # [c, p, b]
    blocks = []
    for bt0, bt1 in PASS_BTS:
        blk = cm[:, :, bt0 * P:bt1 * P]  # [c, p, w]
        blocks.append(blk.transpose(1, 0, 2).reshape(P, -1))
    idxs = np.ascontiguousarray(np.concatenate(blocks, axis=1))
    # lhs bytes carry 1.5*x (the stationary is pre-divided by 1.5)
    xd = LHS * x_core[:, :N_DENSE].T.astype(np.float32)   # [13, bc]
    xhi = _q8(xd)
    xres = _q8(xd - xhi)
    xdn = np.zeros((SP0, bc), np.float32)
    xdn[0] = LHS
    xdn[1:1 + N_DENSE] = xhi
    xdn[14:14 + N_DENSE] = xres
    # pack each e4m3 byte into the HIGH byte of an fp16 lane: the device
    # copies fp16 (fast path) and the matmul reads the high bytes stride-2
    bits = xdn.astype(E4NP).view(np.uint8).astype(np.uint16) << 8
    return idxs, bits.view(np.float16)


def run(x, w_weight, w_bias, v, trace=False, trace_kwargs=None):
    x = np.asarray(x, np.float32)
    w_weight = np.asarray(w_weight, np.float32)
    w_bias = np.asarray(w_bias, np.float32)
    v = np.asarray(v, np.float32)
    assert x.shape == (B_FULL, 39), x.shape

    vp8, off, s_of_r, in_sparse = _prep_shared(w_weight, w_bias, v)
    in_maps = []
    for i in range(N_CORES):
        xc = x[i * BC:(i + 1) * BC]
        idxs, xdn = _prep_core(xc, off, s_of_r, in_sparse)
        in_maps.append({
            "idxs": idxs,
            "xdn": xdn,
            "vperm": vp8,
        })

    nc = _get_program(BC)
    res = run_bass_kernel_spmd(
        nc, in_maps, list(range(N_CORES)),
        trace=trace, **(trace_kwargs or {}),
    )
    y = np.concatenate(
        [res.results[i]["y"].T.reshape(-1, 1) for i in range(N_CORES)], axis=0
    )
    return y.astype(np.float32), res


def kernel(x, w_weight, w_bias, v):
    y, _ = run(x, w_weight, w_bias, v)
    return y
